# revision 5
# baseline (speedup 1.0000x reference)
"""ConvLSTMCellSpike Trainium2 kernel.

Sharding: data-parallel over batch, B=8 -> 1 batch element per NeuronCore
(8 cores), weights replicated; the 32-step recurrence runs fully on-core
with no cross-device traffic.

Layouts (per core):
- "store" [128, 1232]: zero-padded 34x34 plane flattened per channel with
  margins; conceptual padded index q at col q+35.  Rows 64-127 duplicate
  rows 0-63 so a K=128 matmul contracts hi+lo split weights in one pass.
- if-gates (i,f = 128 out-channels) live flat [128, 1089]: conv-output
  flat col f = 34*h + w + 1 for interior (h, w); PSUM [128, 1536] with
  matmul subs (0,512),(512,512),(1024,65) so no bank is crossed.
- g-gate (64 channels) lives COMPACT [128, 546]: partition c+64*half,
  half 0 = plane rows 0-15 (flat [0,545)), half 1 = rows 16-31 (flat
  [545,1090)); local col j -> psum bank j//273, bank col j%273.  This
  halves the free size of every g elementwise op (DVE bills free size).
- conv2d = 9 shifted-tap matmuls accumulated in PSUM; steady-state taps
  are bf16 with hi/lo split weights stacked on K (rows 0-63 hi, 64-127
  lo against duplicated x; x is binary so products are exact, conv
  matches fp32 to ~2^-17).
- whc + biases (pre-scaled 0.1) are added into the conv PSUM as identity
  matmuls of a bf16 multi-level split.

Engine placement per step (~10.2us PE, ~9.5us DVE, ~2.1us Act):
- PE: 18 tap matmuls (9 if-subs x3 merged, 9 g x4 subs) + identity adds.
- DVE: encoder update/reset, v-updates, resets, I-updates (PSUM reads),
  max-tracking.  I tensors are ping-ponged so the I-update (A3/G3) can
  run first and free PSUM before the v ops.
- Act (scalar): spike = Relu(Sign(v_enc - 1)) -> bf16 x store; final
  tanh.  Sign/Relu share one activation table ('small'), no reloads.

Algebraic reductions vs the reference (inherited from earlier work):
- o-gate is the g-gate two steps ahead: vo(t) == vg(t+2) bit-exactly, so
  mo = max(vg[3..33]) and the o-quarter of all weights is dead.
- Synaptic currents stored scaled: I = 0.1*i, so v' = 0.9v + I and
  I' = 0.8I + psum are one fused DVE op each.
- Setup convs (wh, wc on h0/c0) run as 2-pass fp16 hi/lo matmuls:
  passes [Wh;Wl]x[Ah;Al] and [Wh;Wl]x[Al;Ah] recover the fp32 product to
  ~2^-21 at 1 cycle/row instead of fp32's 4 cycles.
"""

import numpy as np

import concourse.bass as bass
import concourse.mybir as mybir
import concourse.tile as tile
from concourse import bacc

F32 = mybir.dt.float32
BF16 = mybir.dt.bfloat16
FP16 = mybir.dt.float16
AL = mybir.AluOpType
ACT = mybir.ActivationFunctionType

STORE = 1232         # padded input plane + margins (col q+35 for padded q)
PW = 1089            # if-gate flat plane width (last interior col 1088)
PWT = 1092           # if state tile width (3 junk tail cols)
GW = 546             # g compact width (2 x 273)
GH = 273             # g psum sub width
SEQ = 32
N_CORES = 8
A_MEM = float(np.float32(0.001 * 100.0))   # 0.1
TAP_OFF = [-35, -34, -33, -1, 0, 1, 33, 34, 35]  # (dh-1)*34 + (dw-1)
IF_SUBS = [(0, 512), (512, 512), (1024, 65)]
WB_LEVELS = 2        # whc identity-add fp16 split levels (2 = ~2^-22)


def _bf(x):
    import ml_dtypes
    return np.asarray(x, np.float32).astype(ml_dtypes.bfloat16)


def _fh(x):
    return np.asarray(x, np.float32).astype(np.float16)


def _split_levels16(w, n):
    """fp32 -> list of n fp16 arrays summing to w (residual splitting)."""
    w = np.asarray(w, np.float32)
    out = []
    r = w
    for _ in range(n):
        h = _fh(r)
        out.append(h)
        r = (r - h.astype(np.float32)).astype(np.float32)
    return out


def _split_levels(w, n):
    """fp32 -> list of n bf16 arrays summing to w (residual splitting)."""
    w = np.asarray(w, np.float32)
    out = []
    r = w
    for _ in range(n):
        h = _bf(r)
        out.append(h)
        r = (r - h.astype(np.float32)).astype(np.float32)
    return out


def _prep_taps_split(Wt, lo_ch, hi_ch, scale, fp16=False):
    """[Cout,64,3,3] -> [128, 9, M]: rows 0-63 hi, rows 64-127 lo of
    scale*W tap k = 3*dh+dw, transposed [Cin, M]."""
    M = hi_ch - lo_ch
    Ws = (np.asarray(Wt[lo_ch:hi_ch], np.float32) * np.float32(scale))
    import ml_dtypes
    dt_ = np.float16 if fp16 else ml_dtypes.bfloat16
    split = _split_levels16 if fp16 else _split_levels
    outb = np.zeros((128, 9, M), dt_)
    for dh in range(3):
        for dw in range(3):
            k = 3 * dh + dw
            levels = split(Ws[:, :, dh, dw].T, 2)
            outb[0:64, k, :] = levels[0]
            outb[64:128, k, :] = levels[1]
    return outb


def _plane_flat(a):
    """[64, 32, 32] -> [64, 1120] flat plane: interior (h,w) at 34h+w+1."""
    f = np.zeros((64, 1120), np.float32)
    v = f[:, :1088].reshape(64, 32, 34)
    v[:, :, 1:33] = a
    return f


def _to_store_pair(a, lvl_lo, lvl_hi):
    """[64,32,32] fp32 -> [128, STORE] fp16 store: rows 0-63 = fp16-split
    level lvl_lo of a, rows 64-127 = level lvl_hi."""
    levels = _split_levels16(a, 2)
    s = np.zeros((128, STORE), np.float32)
    conc = np.zeros((64, 34, 34), np.float32)
    conc[:, 1:33, 1:33] = levels[lvl_lo].astype(np.float32)
    s[0:64, 35:35 + 1156] = conc.reshape(64, 1156)
    conc[:, 1:33, 1:33] = levels[lvl_hi].astype(np.float32)
    s[64:128, 35:35 + 1156] = conc.reshape(64, 1156)
    return _fh(s)


def _to_store_f32(a):
    """[64,32,32] -> [128, STORE] fp32, duplicated halves."""
    s = np.zeros((128, STORE), np.float32)
    conc = np.zeros((64, 34, 34), np.float32)
    conc[:, 1:33, 1:33] = np.asarray(a, np.float32)
    flat = conc.reshape(64, 1156)
    s[0:64, 35:35 + 1156] = flat
    s[64:128, 35:35 + 1156] = flat
    return s


def _to_compact(a):
    """[64,32,32] -> [128, GW] fp32 compact g layout."""
    f = _plane_flat(a)
    out = np.zeros((128, GW), np.float32)
    out[0:64, 0:545] = f[:, 0:545]
    out[64:128, 0:545] = f[:, 545:1090]
    return out


def build_program(nc, repeats=1):
    dp = nc.declare_dram_parameter
    input_s = dp("input_s", [128, STORE], F32, isOutput=False)  # 0.1*input dup
    h0_p1 = dp("h0_p1", [128, STORE], FP16, isOutput=False)     # [Ah;Al]
    h0_p2 = dp("h0_p2", [128, STORE], FP16, isOutput=False)     # [Al;Ah]
    c0_p1 = dp("c0_p1", [128, STORE], FP16, isOutput=False)
    c0_p2 = dp("c0_p2", [128, STORE], FP16, isOutput=False)
    c0c_in = dp("c0c", [128, GW], F32, isOutput=False)
    wih_if = dp("wih_if", [128, 9, 128], BF16, isOutput=False)
    wih_g = dp("wih_g", [128, 9, 64], BF16, isOutput=False)
    whh_if_a = dp("whh_if_a", [128, 9, 128], FP16, isOutput=False)  # [Wh;Wl]
    wch_if_a = dp("wch_if_a", [128, 9, 128], FP16, isOutput=False)
    whh_g_a = dp("whh_g_a", [128, 9, 64], FP16, isOutput=False)
    identh = dp("identh", [128, 128], FP16, isOutput=False)
    ball = dp("ball", [128, 3], F32, isOutput=False)
    h1o = dp("h1", [64, 1024], F32, isOutput=True)
    c1o = dp("c1", [64, 1024], F32, isOutput=True)

    def iv(t):
        # interior view of an if/store-flat tile: [p, 32, 32], row stride 34
        return t[:, 1:1089].rearrange("p (h w) -> p h w", w=34)[:, :, 0:32]

    def sv(t):
        # interior view of a store tile (base col 70 = flat 1 + 35 - ... )
        return t[:, 70:1158].rearrange("p (h w) -> p h w", w=34)[:, :, 0:32]

    def gv(t):
        # [128, GW] compact tile as [p, 2, GH]
        return t[:, 0:GW].rearrange("p (b n) -> p b n", n=GH)

    def pgv(t):
        # g psum [128, 1024] as [p, 2, GH] (bank stride 512)
        return t[:].rearrange("p (b n) -> p b n", n=512)[:, :, 0:GH]

    with tile.TileContext(nc) as tc:
        with (
            tc.tile_pool(name="const", bufs=1) as cpool,
            tc.tile_pool(name="state", bufs=1) as spool,
            tc.tile_pool(name="tmp", bufs=1) as tpool,
            tc.tile_pool(name="psum", bufs=1, space="PSUM") as ppool,
        ):
          # `repeats` re-emits the full body against the same tile slots;
          # tile's dependency tracking serializes iterations.  Used by the
          # timing harness to measure per-invocation device time as a slope.
          for _rep in range(repeats):
            def ctile(shape, name, src, dt_=F32):
                t = cpool.tile(shape, dt_, tag=name)
                nc.sync.dma_start(t[:], src[:])
                return t

            # setup-conv inputs first so PE can start early
            t_h0p, t_c0p = [], []
            for i, s in enumerate((h0_p1, h0_p2)):
                th = ctile([128, STORE], f"h0p{i}", s, FP16)
                t_h0p.append(th)
            for i, s in enumerate((c0_p1, c0_p2)):
                tcp = ctile([128, STORE], f"c0p{i}", s, FP16)
                t_c0p.append(tcp)
            w_hh_if = ctile([128, 9, 128], "whhif", whh_if_a, FP16)
            w_ch_if = ctile([128, 9, 128], "wchif", wch_if_a, FP16)
            w_hh_g = ctile([128, 9, 64], "whhg", whh_g_a, FP16)
            id_h = ctile([128, 128], "identh", identh, FP16)
            ball_t = ctile([128, 3], "ball", ball)
            bsum_if = ball_t[:, 0:1]     # 0.1*(b_ih+b_hh+b_ch)[0:128]
            bsum_g = ball_t[:, 1:2]      # 0.1*(b_ih+b_hh)[g], both halves
            bneg1 = ball_t[:, 2:3]       # -1.0
            w_ih_if = ctile([128, 9, 128], "wih_if", wih_if, BF16)
            w_ih_g = ctile([128, 9, 64], "wih_g", wih_g, BF16)
            in_s = ctile([128, STORE], "input_s", input_s)
            c0c = ctile([128, GW], "c0c", c0c_in)

            # ---- states ----
            venc = spool.tile([128, STORE], F32, tag="venc")
            sgn = spool.tile([128, 1024], F32, tag="sgn")
            xb0 = spool.tile([128, STORE], BF16, tag="xb0")
            xb1 = spool.tile([128, STORE], BF16, tag="xb1")
            vif = spool.tile([128, PWT], F32, tag="vif")
            iifA = spool.tile([128, PWT], F32, tag="iifA")
            iifB = spool.tile([128, PWT], F32, tag="iifB")
            mif = spool.tile([128, PWT], F32, tag="mif")
            vg = spool.tile([128, GW], F32, tag="vg")
            giA = spool.tile([128, GW], F32, tag="giA")
            giB = spool.tile([128, GW], F32, tag="giB")
            mvg = spool.tile([128, GW], F32, tag="mvg")
            vg1 = spool.tile([128, GW], F32, tag="vg1")
            wb_if = []
            wb_g = []
            for i in range(WB_LEVELS):
                lvl_if = spool.tile([128, PWT], FP16, tag=f"wb_if{i}")
                lvl_g = spool.tile([128, GW], FP16, tag=f"wb_g{i}")
                wb_if.append(lvl_if)
                wb_g.append(lvl_g)

            for t in (venc, vif, iifA, iifB, mif):
                nc.gpsimd.memset(t[:], 0.0)
            for t in (vg, giA, giB, mvg, vg1):
                nc.gpsimd.memset(t[:], 0.0)
            nc.gpsimd.memset(xb0[:], 0.0)   # spike writes only the interior
            nc.gpsimd.memset(xb1[:], 0.0)

            # ---- setup: whcb = 0.1*(wh + wc_peephole) + 0.1*biases ----
            # 3-pass bf16 hi/lo conv of fp32 data (see module docstring).
            psA = ppool.tile([128, 1536], F32, tag="psI_B")
            for b, (base, n) in enumerate(IF_SUBS):
                total = 2 * 2 * 9
                i = 0
                for wgt, srcs in ((w_hh_if, t_h0p), (w_ch_if, t_c0p)):
                    for src in srcs:
                        for k in range(9):
                            sig = 69 + base + TAP_OFF[k]
                            nc.tensor.matmul(
                                psA[:, base:base + n], wgt[:, k, :],
                                src[:, sig:sig + n],
                                start=(i == 0), stop=(i == total - 1))
                            i += 1
            whcb_if_f = tpool.tile([128, PWT], F32, tag="whcb_if_f")
            nc.vector.tensor_scalar(whcb_if_f[:, 0:PW], psA[:, 0:PW],
                                    float(2.0 ** -6), bsum_if,
                                    AL.mult, AL.add)

            psG = ppool.tile([128, 1024], F32, tag="psG")
            for h in range(2):
                for s in range(2):
                    out = psG[64 * h:64 * h + 64, 512 * s:512 * s + GH]
                    i = 0
                    for src in t_h0p:
                        for k in range(9):
                            sig = 69 + 545 * h + GH * s + TAP_OFF[k]
                            nc.tensor.matmul(out, w_hh_g[:, k, :],
                                             src[:, sig:sig + GH],
                                             start=(i == 0), stop=(i == 17))
                            i += 1
            whcb_g_f = tpool.tile([128, GW], F32, tag="whcb_g_f")
            nc.vector.tensor_scalar(gv(whcb_g_f), pgv(psG),
                                    float(2.0 ** -6), bsum_g,
                                    AL.mult, AL.add)

            # split whcb into bf16 levels for the per-step identity adds
            def split_levels(levels, srcf, width):
                r = tpool.tile([128, PWT], F32, tag="r_split")
                cur = srcf
                for i, lvl in enumerate(levels):
                    nc.vector.tensor_copy(lvl[:, 0:width], cur[:, 0:width])
                    if i + 1 < len(levels):
                        nxt = tpool.tile([128, PWT], F32,
                                         tag=f"r_split{i % 2}")
                        nc.vector.scalar_tensor_tensor(
                            nxt[:, 0:width], lvl[:, 0:width], -1.0,
                            cur[:, 0:width], AL.mult, AL.add)
                        cur = nxt
            split_levels(wb_if, whcb_if_f, PW)
            split_levels(wb_g, whcb_g_f, GW)

            # ---- steady-state step ----
            xbufs = [xb0, xb1]

            def emit_enc_head(i):
                # encoder membrane update for step i + spike (Act engine)
                vw = sv(venc)
                nc.vector.scalar_tensor_tensor(vw, vw, 0.9, sv(in_s),
                                               AL.mult, AL.add)
                sgv = sgn[:].rearrange("p (h w) -> p h w", w=32)
                nc.scalar.activation(sgv, vw, ACT.Sign, bias=bneg1)
                nc.scalar.activation(sv(xbufs[i % 2]), sgv, ACT.Relu)

            def emit_enc_tail():
                vw = sv(venc)
                nc.vector.scalar_tensor_tensor(vw, vw, 1.0, vw,
                                               AL.is_le, AL.mult)

            emit_enc_head(0)
            emit_enc_tail()
            for t in range(SEQ):
                xb = xbufs[t % 2]
                icur, inxt = (iifA, iifB) if t % 2 == 0 else (iifB, iifA)
                gcur, gnxt = (giA, giB) if t % 2 == 0 else (giB, giA)

                # PE: if-conv (taps + whc identity levels)
                psI = ppool.tile([128, 1536],
                                 F32, tag="psI_A" if t % 2 == 0 else "psI_B")
                for base, n in IF_SUBS:
                    for k in range(9):
                        sig = 69 + base + TAP_OFF[k]
                        nc.tensor.matmul(psI[:, base:base + n],
                                         w_ih_if[:, k, :],
                                         xb[:, sig:sig + n],
                                         start=(k == 0), stop=False)
                    for j, lvl in enumerate(wb_if):
                        nc.tensor.matmul(psI[:, base:base + n],
                                         id_h[:, 0:128],
                                         lvl[:, base:base + n],
                                         start=False,
                                         stop=(j == WB_LEVELS - 1))
                # PE: g-conv (compact layout)
                psG2 = ppool.tile([128, 1024], F32, tag="psG")
                for s in range(2):
                    for h in range(2):
                        out = psG2[64 * h:64 * h + 64,
                                   512 * s:512 * s + GH]
                        for k in range(9):
                            sig = 69 + 545 * h + GH * s + TAP_OFF[k]
                            nc.tensor.matmul(out, w_ih_g[:, k, :],
                                             xb[:, sig:sig + GH],
                                             start=(k == 0), stop=False)
                    for j, lvl in enumerate(wb_g):
                        nc.tensor.matmul(psG2[:, 512 * s:512 * s + GH],
                                         id_h[:, 0:128],
                                         gv(lvl)[:, s, :],
                                         start=False,
                                         stop=(j == WB_LEVELS - 1))

                # encoder for step t+1 (DVE head op + Act spike)
                if t + 1 < SEQ:
                    emit_enc_head(t + 1)

                # DVE: I-updates first (free PSUM), then v-updates
                nc.vector.scalar_tensor_tensor(
                    inxt[:, 0:PW], icur[:, 0:PW], 0.8, psI[:, 0:PW],
                    AL.mult, AL.add)
                nc.vector.scalar_tensor_tensor(
                    gv(gnxt), gv(gcur), 0.8, pgv(psG2), AL.mult, AL.add)
                nc.vector.scalar_tensor_tensor(iv(vif), iv(vif), 0.9,
                                               iv(icur), AL.mult, AL.add)
                nc.vector.scalar_tensor_tensor(iv(vif), iv(vif), 1.0,
                                               iv(vif), AL.is_le, AL.mult)
                nc.vector.scalar_tensor_tensor(gv(vg), gv(vg), 0.9,
                                               gv(gcur), AL.mult, AL.add)
                nc.vector.scalar_tensor_tensor(gv(vg), gv(vg), 1.0,
                                               gv(vg), AL.is_le, AL.mult)

                if t == 1:
                    nc.vector.tensor_copy(iv(mif), iv(vif))
                    nc.vector.tensor_copy(vg1[:], vg[:])
                elif t == 2:
                    nc.vector.tensor_max(iv(mif), iv(mif), iv(vif))
                    nc.vector.tensor_copy(mvg[:], vg[:])
                elif t >= 3:
                    nc.vector.tensor_max(iv(mif), iv(mif), iv(vif))
                    nc.vector.tensor_max(mvg[:], mvg[:], vg[:])

                # encoder reset last (Sign has long finished)
                if t + 1 < SEQ:
                    emit_enc_tail()

            # ---- epilogue (all in compact g layout) ----
            gfin = giA if SEQ % 2 == 0 else giB
            # one more g membrane update: vg(33) for the o-gate max
            nc.vector.scalar_tensor_tensor(gv(vg), gv(vg), 0.9, gv(gfin),
                                           AL.mult, AL.add)
            nc.vector.scalar_tensor_tensor(gv(vg), gv(vg), 1.0, gv(vg),
                                           AL.is_le, AL.mult)
            mo_s = tpool.tile([128, GW], F32, tag="mo_s")
            mg_s = tpool.tile([128, GW], F32, tag="mg_s")
            nc.vector.tensor_max(mo_s[:], mvg[:], vg[:])    # max vg[3..33]
            nc.vector.tensor_max(mg_s[:], mvg[:], vg1[:])   # max vg[2..32]

            # mi/mf into compact layout (SBUF->SBUF DMA, partition remap)
            mi_c = tpool.tile([128, GW], F32, tag="mi_c")
            mf_c = tpool.tile([128, GW], F32, tag="mf_c")
            for dst, lo in ((mi_c, 0), (mf_c, 64)):
                nc.sync.dma_start(dst[0:64, :], mif[lo:lo + 64, 0:GW])
                nc.sync.dma_start(dst[64:128, :],
                                  mif[lo:lo + 64, 545:545 + GW])
            ta = tpool.tile([128, GW], F32, tag="ta")
            nc.vector.tensor_mul(ta[:], mf_c[:], c0c[:])
            tb2 = tpool.tile([128, GW], F32, tag="tb2")
            nc.vector.tensor_mul(tb2[:], mi_c[:], mg_s[:])
            c1s = tpool.tile([128, GW], F32, tag="c1s")
            nc.vector.tensor_add(c1s[:], ta[:], tb2[:])
            ths = tpool.tile([128, GW], F32, tag="ths")
            nc.scalar.activation(ths[:], c1s[:], ACT.Tanh)
            h1s = tpool.tile([128, GW], F32, tag="h1s")
            nc.vector.tensor_mul(h1s[:], mo_s[:], ths[:])

            # outputs: interior of each half
            def out_half(dst, src):
                h0v = src[0:64, 1:545].rearrange(
                    "c (h w) -> c h w", w=34)[:, :, 0:32]
                h1v = src[64:128, 0:544].rearrange(
                    "c (h w) -> c h w", w=34)[:, :, 0:32]
                dv = dst[:].rearrange("c (h w) -> c h w", w=32)
                nc.sync.dma_start(dv[:, 0:16, :], h0v)
                nc.sync.dma_start(dv[:, 16:32, :], h1v)
            out_half(h1o, h1s)
            out_half(c1o, c1s)
    return nc


def make_in_maps(input, h0, c0, weight_ih, weight_hh, weight_ch,
                 bias_ih, bias_hh, bias_ch):
    f = lambda a: np.ascontiguousarray(np.asarray(a, np.float32))
    input, h0, c0 = f(input), f(h0), f(c0)
    weight_ih, weight_hh, weight_ch = f(weight_ih), f(weight_hh), f(weight_ch)
    bias_ih, bias_hh, bias_ch = f(bias_ih), f(bias_hh), f(bias_ch)
    a_m = np.float32(A_MEM)

    shared = dict(
        wih_if=_prep_taps_split(weight_ih, 0, 128, a_m),
        wih_g=_prep_taps_split(weight_ih, 128, 192, a_m),
        whh_if_a=_prep_taps_split(weight_hh, 0, 128, a_m * 4096.0, fp16=True),
        wch_if_a=_prep_taps_split(weight_ch, 0, 128, a_m * 4096.0, fp16=True),
        whh_g_a=_prep_taps_split(weight_hh, 128, 192, a_m * 4096.0, fp16=True),
        identh=_fh(np.eye(128) / 64.0),
    )
    # biases pre-scaled by 64: whcb tiles hold 64*whc (fp16-split range),
    # compensated by the 2^-6 identity in the per-step id matmuls
    ball = np.zeros((128, 3), np.float32)
    ball[:, 0] = 64.0 * a_m * (bias_ih[0:128] + bias_hh[0:128] + bias_ch[0:128])
    bg = 64.0 * a_m * (bias_ih[128:192] + bias_hh[128:192])
    ball[0:64, 1] = bg
    ball[64:128, 1] = bg
    ball[:, 2] = -1.0
    shared["ball"] = ball

    in_maps = []
    for b in range(N_CORES):
        m = dict(shared)
        m["input_s"] = _to_store_f32(input[b] * a_m)
        m["h0_p1"] = _to_store_pair(h0[b], 0, 1)   # [Ah;Al]
        m["h0_p2"] = _to_store_pair(h0[b], 1, 0)   # [Al;Ah]
        m["c0_p1"] = _to_store_pair(c0[b], 0, 1)
        m["c0_p2"] = _to_store_pair(c0[b], 1, 0)
        m["c0c"] = _to_compact(c0[b])
        in_maps.append(m)
    return in_maps


_PROG = {}


def get_program(repeats=1):
    if repeats not in _PROG:
        nc = bacc.Bacc("TRN2", target_bir_lowering=False, debug=False)
        build_program(nc, repeats=repeats)
        nc.compile()
        _PROG[repeats] = nc
    return _PROG[repeats]


def kernel(**inputs):
    from concourse.bass_utils import run_bass_kernel_spmd

    nc = get_program()
    in_maps = make_in_maps(**inputs)
    res = run_bass_kernel_spmd(nc, in_maps, list(range(N_CORES)))
    h1 = np.stack([res.results[b]["h1"].reshape(64, 32, 32)
                   for b in range(N_CORES)])
    c1 = np.stack([res.results[b]["c1"].reshape(64, 32, 32)
                   for b in range(N_CORES)])
    return (h1, c1)
